# revision 13
# baseline (speedup 1.0000x reference)
"""Trainium2 Bass kernel for nn_DynaResidualBlockX (hypernet + per-sample 1x1 conv residual block).

Strategy (8 NeuronCores), v3:
  - Hypernet `ks = lat @ W.T + b` sharded by W rows: each core computes 1/8 of
    the per-sample conv kernels for ALL 16 samples (reads 1/8 of W, permuted/
    padded on the host into the "W2" tile layout the tensor engine wants).
  - Three pipelined AllToAlls (G1 = kin+b_in, G2 = mida+b_mida, G3 = rest)
    hand core c the kernel set for its 2 samples. Exchange payload is staged
    SAMPLE-MAJOR so every post-exchange weight slice is contiguous (fast DMAs).
  - x loaded once, packed [s0 64ch | s1 64ch] = [128, HWP]. L1 runs the two
    samples CONCURRENTLY in the PE via row-group tiling (s0 weights at
    partitions 0-63, s1 at 64-127). L4 computes the k_short residual for both
    samples in one pass via a block-diagonal lhsT.
  - Biases: b_in/b_mida/b_midb applied in the relu ops; b_out+b_short applied
    on the HOST (outside all nonlinearities). Output stored bf16.
  - Conv is software-pipelined chunk-granular: round j runs L1(j), L2(j-1),
    L3(j-2), L4(j-3) so the PE never waits on a relu; relu/copy ops are
    weight-balanced between DVE and ACT.
"""

import os
import sys

if "/opt/trn_rl_repo" not in sys.path:
    sys.path.insert(0, "/opt/trn_rl_repo")

import numpy as np

# ---------------- problem constants (hardcoded per contract) ----------------
B, FIN, FOUT, FH, LAT = 16, 64, 64, 128, 512
HWP = 128 * 128  # pixels per image
NCORE, BC = 8, 2  # cores, samples per core
GT = [136, 136, 264]  # tiles per group (8-divisible)
GPC = [g // NCORE for g in GT]  # tiles per core per group: 17, 17, 33
TILES = sum(GT)  # 536
KT2 = TILES * 128
GOFF = [0, GT[0], GT[0] + GT[1]]
JP = 2048  # conv pixel chunk
NJ = HWP // JP  # 8 chunks
NP = 1024  # psum tile / act-op granularity
MMN = 512  # matmul moving free-dim (PSUM bank limit)
GCHUNKS = [[9, 8], [9, 8], [9, 8, 8, 8]]  # hypernet W-chunk tile counts

# tile indices within groups
G1_BIN = 128
G2_BMIDA = 128
G3_MIDB, G3_KOUT, G3_KSH, G3_BMIDB = 0, 128, 192, 256

S128 = 1.0 / np.sqrt(128.0)
S64 = 1.0 / 8.0


def _np_bf16():
    import ml_dtypes

    return ml_dtypes.bfloat16


def _build_w2b2(W, b):
    """Permute/pad/scale hypernet weights into the device tile layout."""
    r = np.arange(KT2)
    t, p = r >> 7, r & 127
    src = np.full(KT2, -1, np.int64)
    scale = np.ones(KT2, np.float32)

    m = (t < 128) & (p < 64)  # G1 kin (t = fh, p = fi)
    src[m] = t[m] * 64 + p[m]
    scale[m] = S128
    m = t == G1_BIN
    src[m] = 53248 + p[m]
    u = t - GOFF[1]  # G2 mida (u = fh_out, p = fh_in)
    m = (u >= 0) & (u < 128)
    src[m] = 8192 + u[m] * 128 + p[m]
    scale[m] = S128
    m = u == G2_BMIDA
    src[m] = 53376 + p[m]
    w = t - GOFF[2]  # G3: midb, kout, kshort, b_midb
    m = (w >= 0) & (w < 128)
    src[m] = 24576 + w[m] * 128 + p[m]
    scale[m] = S128
    m = (w >= 128) & (w < 192)
    src[m] = 40960 + (w[m] - 128) * 128 + p[m]
    scale[m] = S64
    m = (w >= 192) & (w < 256) & (p < 64)
    src[m] = 49152 + (w[m] - 192) * 64 + p[m]
    scale[m] = S64
    m = w == G3_BMIDB
    src[m] = 53504 + p[m]

    W2 = np.zeros((KT2, LAT), np.float32)
    b2 = np.zeros(KT2, np.float32)
    v = src >= 0
    W2[v] = W[src[v]] * scale[v][:, None]
    b2[v] = b[src[v]] * scale[v]
    return W2, b2


def _host_bos(lat, W, b):
    """b_out + b_short per (sample, channel) — applied on the host."""
    Wos = W[53632:53696] + W[53696:53760]
    bos = b[53632:53696] + b[53696:53760]
    return np.asarray(lat, np.float32) @ Wos.T.astype(np.float32) + bos


def _host_inputs(x, lat, W, b):
    """Build the 8 per-core input maps (pure layout work, no math)."""
    ndt = _np_bf16()
    x = np.ascontiguousarray(x, np.float32).reshape(B, FIN, HWP)
    lat = np.ascontiguousarray(lat, np.float32)
    W2, b2 = _build_w2b2(np.asarray(W, np.float32), np.asarray(b, np.float32))

    latt = np.ascontiguousarray(
        lat.T.reshape(4, 128, 16).transpose(1, 0, 2).reshape(128, 64).astype(ndt)
    )

    b2d = {}
    for g in range(3):
        lo = GOFF[g] * 128
        mat = b2[lo : lo + GT[g] * 128].reshape(GT[g], 128).T  # [128, GT]
        b2d[g] = np.ascontiguousarray(  # sample-major duplicate
            np.concatenate([mat, mat], axis=1).astype(ndt)
        )

    def wshard(g, c):  # -> [128, GPC[g]*512] in [q, t, l, m] order
        ksg = GPC[g] * 128
        lo = GOFF[g] * 128 + c * ksg
        sh = W2[lo : lo + ksg]
        return (
            sh.reshape(GPC[g], 128, 4, 128)
            .transpose(3, 0, 2, 1)
            .reshape(128, GPC[g] * 512)
        )

    in_maps = []
    for c in range(NCORE):
        wmain = np.ascontiguousarray(
            np.concatenate([wshard(g, c) for g in range(3)], axis=1).astype(ndt)
        )
        xp = np.ascontiguousarray(
            x[c * BC : (c + 1) * BC].reshape(BC * FIN, HWP).astype(ndt)
        )
        in_maps.append(
            {
                "wmain": wmain,
                "latt": latt,
                "b2dup1": b2d[0],
                "b2dup2": b2d[1],
                "b2dup3": b2d[2],
                "xs": xp,
            }
        )
    return in_maps


def _post(raw_all, lat, W, b):
    """raw device outs (list of [BC*FOUT, HWP]) -> full f32 output."""
    out = np.concatenate([np.asarray(r, np.float32) for r in raw_all], axis=0)
    out = out.reshape(B, FOUT, HWP)
    out += _host_bos(lat, W, b)[:, :, None]
    return np.ascontiguousarray(out.reshape(B, FOUT, 128, 128))


def emulate(x, lat, W, b):
    """Numpy emulation of the device dataflow (for layout validation)."""
    xf = np.asarray(x, np.float32).reshape(B, FIN, HWP)
    W2, b2 = _build_w2b2(np.asarray(W, np.float32), np.asarray(b, np.float32))
    ksT = W2 @ np.asarray(lat, np.float32).T + b2[:, None]
    v1 = ksT[: GT[0] * 128].reshape(GT[0], 128, B)
    v2 = ksT[GT[0] * 128 : GOFF[2] * 128].reshape(GT[1], 128, B)
    v3 = ksT[GOFF[2] * 128 :].reshape(GT[2], 128, B)
    raws = []
    for c in range(NCORE):
        raw = np.zeros((BC * FOUT, HWP), np.float32)
        for s in range(BC):
            bi = c * BC + s
            xb = xf[bi]
            h1 = np.maximum(v1[0:128, 0:64, bi] @ xb + v1[G1_BIN, :, bi][:, None], 0.0)
            h2 = np.maximum(v2[0:128, :, bi] @ h1 + v2[G2_BMIDA, :, bi][:, None], 0.0)
            h3 = np.maximum(
                v3[0:128, :, bi] @ h2 + v3[G3_BMIDB, :, bi][:, None], 0.0
            )
            a_out = v3[G3_KOUT : G3_KOUT + 64, :, bi]
            a_sh = v3[G3_KSH : G3_KSH + 64, 0:64, bi]
            raw[s * 64 : (s + 1) * 64] = a_out @ h3 + a_sh @ xb
        raws.append(raw)
    return _post(raws, lat, W, b)


# ---------------------------- bass program ----------------------------------

def _build_nc():
    import concourse.tile as tile
    from concourse import bacc, mybir

    F32 = mybir.dt.float32
    DT = mybir.dt.bfloat16
    AF = mybir.ActivationFunctionType
    ALU = mybir.AluOpType

    nc = bacc.Bacc("TRN2", target_bir_lowering=False, debug=False, num_devices=NCORE)

    wtot = sum(GPC) * 512
    wmain = nc.dram_tensor("wmain", [128, wtot], DT, kind="ExternalInput")
    latt = nc.dram_tensor("latt", [128, 64], DT, kind="ExternalInput")
    b2d = [
        nc.dram_tensor(f"b2dup{g + 1}", [128, GT[g] * 2], DT, kind="ExternalInput")
        for g in range(3)
    ]
    xs = nc.dram_tensor("xs", [BC * FIN, HWP], DT, kind="ExternalInput")
    outd = nc.dram_tensor("out", [BC * FOUT, HWP], DT, kind="ExternalOutput")

    with tile.TileContext(nc) as tc:
        with (
            tc.tile_pool(name="persist", bufs=1) as pp,
            tc.tile_pool(name="wpool", bufs=3) as wp,
            tc.tile_pool(name="conv", bufs=2) as cp,
            tc.tile_pool(name="ps", bufs=4, space="PSUM") as psp,
            tc.tile_pool(name="dram", bufs=1, space="DRAM") as dp,
        ):
            # ---- phase A: input DMAs (scalar queue: small stuff; gpsimd: x)
            latt_sb = pp.tile([128, 64], DT)
            nc.scalar.dma_start(latt_sb[:], latt[:])
            b2sb = []
            for g in range(3):
                t = pp.tile([128, GT[g] * 2], DT, name=f"b2sb{g}")
                nc.scalar.dma_start(t[:], b2d[g][:])
                b2sb.append(t)
            xpk = pp.tile([128, HWP], DT, name="xpk")
            for i in range(4):
                (nc.gpsimd if i % 2 == 0 else nc.scalar).dma_start(
                    xpk[:, i * 4096 : (i + 1) * 4096],
                    xs[:, i * 4096 : (i + 1) * 4096],
                )

            wm_off = [0]
            for g in range(3):
                wm_off.append(wm_off[-1] + GPC[g] * 512)

            # ---- phase B: hypernet per group -> sample-major staging
            cc_in, cc_out = [], []
            for g in range(3):
                cc_in.append(
                    dp.tile([NCORE, 128, GPC[g] * BC], DT, name=f"cc_in{g}")
                )
                cc_out.append(
                    dp.tile([NCORE, 128, GPC[g] * BC], DT, name=f"cc_out{g}")
                )

            def hyper_group(g):
                # whole-group staging tile, sample-major [p, (d, s, t)]
                ks_stg = pp.tile([128, GPC[g] * 16], DT, name=f"ks_stg{g}")
                stg_v = ks_stg.rearrange(
                    "p (d s t) -> p d s t", d=NCORE, s=BC, t=GPC[g]
                )
                toff = 0
                for ci, csz in enumerate(GCHUNKS[g]):
                    wt = wp.tile([128, 9 * 512], DT, tag="wt", name=f"wt{g}{ci}")
                    lo = wm_off[g] + toff * 512
                    nc.sync.dma_start(
                        wt[:, 0 : csz * 512], wmain[:, lo : lo + csz * 512]
                    )
                    wtv = wt.rearrange("p (t l m) -> p t l m", t=9, l=4)
                    pk = psp.tile([128, csz * 16], F32, tag="ps", name=f"pk{g}{ci}")
                    for tl in range(csz):
                        for l in range(4):
                            nc.tensor.matmul(
                                pk[:, tl * 16 : (tl + 1) * 16],
                                wtv[:, tl, l, :],
                                latt_sb[:, l * 16 : (l + 1) * 16],
                                start=(l == 0),
                                stop=(l == 3),
                            )
                    nc.vector.tensor_copy(
                        stg_v[:, :, :, toff : toff + csz],
                        pk.rearrange("p (t d s) -> p d s t", t=csz, d=NCORE, s=BC),
                    )
                    toff += csz
                nc.sync.dma_start(
                    cc_in[g][:, :, :].rearrange("d p r -> p d r"),
                    stg_v.rearrange("p d s t -> p d (s t)"),
                )

            for g in range(3):
                hyper_group(g)

            # ---- phase C: three pipelined AllToAlls (gpsimd triggers)
            for g in range(3):
                nc.gpsimd.collective_compute(
                    "AllToAll",
                    ALU.bypass,
                    replica_groups=[list(range(NCORE))],
                    ins=[cc_in[g].opt()],
                    outs=[cc_out[g].opt()],
                )

            # Post-exchange work is wrapped in tile_wait_until so the Tile
            # scheduler (whose cost model does not know the ~50us CC-init
            # barrier) cannot hoist it ahead of the staging copies/DMAs that
            # feed later AllToAll triggers — that ordering froze the CC
            # pipeline (trigger blocked -> 135us barrier).
            ksr = []
            for g in range(3):
                ksr.append(pp.tile([128, GT[g] * BC], DT, name=f"ksraw{g}"))
            pksh = pp.tile([128, 128], DT, name="pksh")
            nc.vector.memset(pksh[:], 0.0)
            with tc.tile_wait_until(0.060):
                nc.sync.dma_start(
                    ksr[0].rearrange("p (s c t) -> p s c t", s=BC, c=NCORE),
                    cc_out[0][:, :, :].rearrange("c p (s t) -> p s c t", s=BC),
                )
                ks1 = pp.tile([128, GT[0] * BC], DT, name="ks1")
                nc.vector.tensor_tensor(ks1[:], ksr[0][:], b2sb[0][:], op=ALU.add)
                bin2 = pp.tile([128, 2], F32, name="bin2")
                nc.vector.tensor_copy(bin2[:, 0:1], ks1[:, G1_BIN : G1_BIN + 1])
                nc.vector.tensor_copy(
                    bin2[:, 1:2], ks1[:, GT[0] + G1_BIN : GT[0] + G1_BIN + 1]
                )
                kin1 = pp.tile([128, 128], DT, name="kin1")
                nc.gpsimd.dma_start(
                    kin1[64:128, 0:128], ks1[0:64, GT[0] : GT[0] + 128]
                )
            with tc.tile_wait_until(0.068):
                nc.gpsimd.dma_start(
                    ksr[1].rearrange("p (s c t) -> p s c t", s=BC, c=NCORE),
                    cc_out[1][:, :, :].rearrange("c p (s t) -> p s c t", s=BC),
                )
                ks2 = pp.tile([128, GT[1] * BC], DT, name="ks2")
                nc.gpsimd.tensor_tensor(ks2[:], ksr[1][:], b2sb[1][:], op=ALU.add)
                bma2 = pp.tile([128, 2], F32, name="bma2")
                nc.gpsimd.tensor_copy(bma2[:, 0:1], ks2[:, G2_BMIDA : G2_BMIDA + 1])
                nc.gpsimd.tensor_copy(
                    bma2[:, 1:2], ks2[:, GT[1] + G2_BMIDA : GT[1] + G2_BMIDA + 1]
                )
            with tc.tile_wait_until(0.076):
                nc.gpsimd.dma_start(
                    ksr[2].rearrange("p (s c t) -> p s c t", s=BC, c=NCORE),
                    cc_out[2][:, :, :].rearrange("c p (s t) -> p s c t", s=BC),
                )
                ks3 = pp.tile([128, GT[2] * BC], DT, name="ks3")
                nc.gpsimd.tensor_tensor(ks3[:], ksr[2][:], b2sb[2][:], op=ALU.add)
                bmb2 = pp.tile([128, 2], F32, name="bmb2")
                nc.gpsimd.tensor_copy(bmb2[:, 0:1], ks3[:, G3_BMIDB : G3_BMIDB + 1])
                nc.gpsimd.tensor_copy(
                    bmb2[:, 1:2], ks3[:, GT[2] + G3_BMIDB : GT[2] + G3_BMIDB + 1]
                )
                nc.gpsimd.dma_start(
                    pksh[0:64, 0:64], ks3[0:64, G3_KSH : G3_KSH + 64]
                )
                nc.gpsimd.dma_start(
                    pksh[64:128, 64:128],
                    ks3[0:64, GT[2] + G3_KSH : GT[2] + G3_KSH + 64],
                )

            # PE-warming chain: keep the tensor engine busy across the
            # exchange-wait gap so HAM holds the 2.4GHz clock when conv
            # starts. WAW on one psum region serializes the matmuls; results
            # are never used.
            with tc.tile_wait_until(0.0605):
                pwarm = psp.tile([64, MMN], F32, tag="ps", name="pwarm")
                for i in range(45):
                    nc.tensor.matmul(
                        pwarm[:],
                        latt_sb[:, 0:64],
                        xpk[:, 0:MMN],
                        start=True,
                        stop=True,
                    )
                wsink = pp.tile([64, 16], F32, name="wsink")
                nc.vector.tensor_copy(wsink[:], pwarm[:, 0:16])

            # ---- phase D: conv, chunk-granular software pipeline
            ta, td = [0.0], [0.0]

            def act_op(dst, src, bias, kind, n):
                ca, cd = 0.175 + n / 1.2e3, 0.170 + n / 0.96e3
                if ta[0] + ca <= td[0] + cd:
                    ta[0] += ca
                    if kind == "relu":
                        nc.scalar.activation(dst, src, AF.Relu, bias=bias)
                    else:
                        nc.scalar.activation(dst, src, AF.Copy)
                else:
                    td[0] += cd
                    if kind == "relu":
                        nc.vector.tensor_scalar(
                            dst, src, bias, 0.0, op0=ALU.add, op1=ALU.max
                        )
                    else:
                        nc.vector.tensor_copy(dst, src)

            def layer(dst_h, lhsT, rhs_tile, rows, cols0, bias):
                for half in range(JP // NP):
                    ph = psp.tile([128, NP], F32, tag="ps", name=f"ph{half}")
                    for n in range(NP // MMN):
                        lo = cols0 + half * NP + n * MMN
                        nc.tensor.matmul(
                            ph[:, n * MMN : (n + 1) * MMN],
                            lhsT,
                            rhs_tile[rows, lo : lo + MMN],
                            start=True,
                            stop=True,
                        )
                    hsl = slice(half * NP, (half + 1) * NP)
                    act_op(dst_h[:, hsl], ph[:, 0:NP], bias, "relu", NP)

            HB2 = 10
            h1s, h2s, h3s = {}, {}, {}

            def do_L1(j):
                for s in range(BC):
                    h1 = cp.tile([128, JP], DT, tag="h", bufs=HB2, name=f"h1_{s}")
                    if s == 0:
                        layer(h1, ks1[0:64, 0:128], xpk, slice(0, 64), j * JP,
                              bin2[:, 0:1])
                    else:
                        layer(h1, kin1[64:128, 0:128], xpk, slice(64, 128), j * JP,
                              bin2[:, 1:2])
                    h1s[j, s] = h1

            def do_L2(j):
                for s in range(BC):
                    h2 = cp.tile([128, JP], DT, tag="h", bufs=HB2, name=f"h2_{s}")
                    layer(h2, ks2[:, s * GT[1] : s * GT[1] + 128], h1s[j, s],
                          slice(0, 128), 0, bma2[:, s : s + 1])
                    h2s[j, s] = h2
                del h1s[j, 0], h1s[j, 1]

            def do_L3(j):
                for s in range(BC):
                    h3 = cp.tile([128, JP], DT, tag="h", bufs=HB2, name=f"h3_{s}")
                    layer(h3, ks3[:, s * GT[2] : s * GT[2] + 128], h2s[j, s],
                          slice(0, 128), 0, bmb2[:, s : s + 1])
                    h3s[j, s] = h3
                del h2s[j, 0], h2s[j, 1]

            def do_L4(j):
                oc = cp.tile([128, JP], DT, tag="oc", bufs=3, name="oc")
                for half in range(JP // NP):
                    po = psp.tile([128, NP], F32, tag="ps", name=f"po{half}")
                    for n in range(NP // MMN):
                        nsl = slice(n * MMN, (n + 1) * MMN)
                        hlo = half * NP + n * MMN
                        nc.tensor.matmul(
                            po[:, nsl],
                            pksh[:],
                            xpk[:, j * JP + hlo : j * JP + hlo + MMN],
                            start=True,
                            stop=False,
                        )
                        nc.tensor.matmul(
                            po[0:64, nsl],
                            ks3[:, G3_KOUT : G3_KOUT + 64],
                            h3s[j, 0][:, hlo : hlo + MMN],
                            start=False,
                            stop=False,
                            tile_position=(0, 0),
                        )
                        nc.tensor.matmul(
                            po[64:128, nsl],
                            ks3[:, GT[2] + G3_KOUT : GT[2] + G3_KOUT + 64],
                            h3s[j, 1][:, hlo : hlo + MMN],
                            start=False,
                            stop=True,
                            tile_position=(0, 64),
                        )
                    act_op(oc[:, half * NP : (half + 1) * NP], po[:, 0:NP],
                           None, "copy", NP)
                nc.gpsimd.dma_start(outd[:, j * JP : (j + 1) * JP], oc[:])
                del h3s[j, 0], h3s[j, 1]

            for r in range(NJ + 3):
                with tc.tile_wait_until(0.062 + 0.004 * r):
                    if r < NJ:
                        do_L1(r)
                    if 0 <= r - 1 < NJ:
                        do_L2(r - 1)
                    if 0 <= r - 2 < NJ:
                        do_L3(r - 2)
                    if 0 <= r - 3 < NJ:
                        do_L4(r - 3)

    nc.compile()
    return nc


_NC_CACHE = None


def kernel(x, lat, W, b):
    from concourse.bass_utils import run_bass_kernel_spmd

    global _NC_CACHE
    if _NC_CACHE is None:
        _NC_CACHE = _build_nc()
    nc = _NC_CACHE
    in_maps = _host_inputs(x, lat, W, b)
    res = run_bass_kernel_spmd(nc, in_maps, core_ids=list(range(NCORE)))
    raws = [res.results[c]["out"] for c in range(NCORE)]
    return _post(raws, lat, W, b)


# revision 15
# speedup vs baseline: 1.0161x; 1.0161x over previous
"""Trainium2 Bass kernel for nn_DynaResidualBlockX (hypernet + per-sample 1x1 conv residual block).

Strategy (8 NeuronCores), v3:
  - Hypernet `ks = lat @ W.T + b` sharded by W rows: each core computes 1/8 of
    the per-sample conv kernels for ALL 16 samples (reads 1/8 of W, permuted/
    padded on the host into the "W2" tile layout the tensor engine wants).
  - Three pipelined AllToAlls (G1 = kin+b_in, G2 = mida+b_mida, G3 = rest)
    hand core c the kernel set for its 2 samples. Exchange payload is staged
    SAMPLE-MAJOR so every post-exchange weight slice is contiguous (fast DMAs).
  - x loaded once, packed [s0 64ch | s1 64ch] = [128, HWP]. L1 runs the two
    samples CONCURRENTLY in the PE via row-group tiling (s0 weights at
    partitions 0-63, s1 at 64-127). L4 computes the k_short residual for both
    samples in one pass via a block-diagonal lhsT.
  - Biases: b_in/b_mida/b_midb applied in the relu ops; b_out+b_short applied
    on the HOST (outside all nonlinearities). Output stored bf16.
  - Conv is software-pipelined chunk-granular: round j runs L1(j), L2(j-1),
    L3(j-2), L4(j-3) so the PE never waits on a relu; relu/copy ops are
    weight-balanced between DVE and ACT.
"""

import os
import sys

if "/opt/trn_rl_repo" not in sys.path:
    sys.path.insert(0, "/opt/trn_rl_repo")

import numpy as np

# ---------------- problem constants (hardcoded per contract) ----------------
B, FIN, FOUT, FH, LAT = 16, 64, 64, 128, 512
HWP = 128 * 128  # pixels per image
NCORE, BC = 8, 2  # cores, samples per core
GT = [136, 136, 264]  # tiles per group (8-divisible)
GPC = [g // NCORE for g in GT]  # tiles per core per group: 17, 17, 33
TILES = sum(GT)  # 536
KT2 = TILES * 128
GOFF = [0, GT[0], GT[0] + GT[1]]
JP = 2048  # conv pixel chunk
NJ = HWP // JP  # 8 chunks
NP = 1024  # psum tile / act-op granularity
MMN = 512  # matmul moving free-dim (PSUM bank limit)
GCHUNKS = [[9, 8], [9, 8], [9, 8, 8, 8]]  # hypernet W-chunk tile counts

# tile indices within groups
G1_BIN = 128
G2_BMIDA = 128
G3_MIDB, G3_KOUT, G3_KSH, G3_BMIDB = 0, 128, 192, 256

S128 = 1.0 / np.sqrt(128.0)
S64 = 1.0 / 8.0


def _np_bf16():
    import ml_dtypes

    return ml_dtypes.bfloat16


def _build_w2b2(W, b):
    """Permute/pad/scale hypernet weights into the device tile layout."""
    r = np.arange(KT2)
    t, p = r >> 7, r & 127
    src = np.full(KT2, -1, np.int64)
    scale = np.ones(KT2, np.float32)

    m = (t < 128) & (p < 64)  # G1 kin (t = fh, p = fi)
    src[m] = t[m] * 64 + p[m]
    scale[m] = S128
    m = t == G1_BIN
    src[m] = 53248 + p[m]
    u = t - GOFF[1]  # G2 mida (u = fh_out, p = fh_in)
    m = (u >= 0) & (u < 128)
    src[m] = 8192 + u[m] * 128 + p[m]
    scale[m] = S128
    m = u == G2_BMIDA
    src[m] = 53376 + p[m]
    w = t - GOFF[2]  # G3: midb, kout, kshort, b_midb
    m = (w >= 0) & (w < 128)
    src[m] = 24576 + w[m] * 128 + p[m]
    scale[m] = S128
    m = (w >= 128) & (w < 192)
    src[m] = 40960 + (w[m] - 128) * 128 + p[m]
    scale[m] = S64
    m = (w >= 192) & (w < 256) & (p < 64)
    src[m] = 49152 + (w[m] - 192) * 64 + p[m]
    scale[m] = S64
    m = w == G3_BMIDB
    src[m] = 53504 + p[m]

    W2 = np.zeros((KT2, LAT), np.float32)
    b2 = np.zeros(KT2, np.float32)
    v = src >= 0
    W2[v] = W[src[v]] * scale[v][:, None]
    b2[v] = b[src[v]] * scale[v]
    return W2, b2


def _host_bos(lat, W, b):
    """b_out + b_short per (sample, channel) — applied on the host."""
    Wos = W[53632:53696] + W[53696:53760]
    bos = b[53632:53696] + b[53696:53760]
    return np.asarray(lat, np.float32) @ Wos.T.astype(np.float32) + bos


def _host_inputs(x, lat, W, b):
    """Build the 8 per-core input maps (pure layout work, no math)."""
    ndt = _np_bf16()
    x = np.ascontiguousarray(x, np.float32).reshape(B, FIN, HWP)
    lat = np.ascontiguousarray(lat, np.float32)
    W2, b2 = _build_w2b2(np.asarray(W, np.float32), np.asarray(b, np.float32))

    latt = np.ascontiguousarray(
        lat.T.reshape(4, 128, 16).transpose(1, 0, 2).reshape(128, 64).astype(ndt)
    )

    def b2pre(g, c):  # [128, GPC*16]: my tile rows' bias, bcast over (d, s)
        lo = (GOFF[g] + c * GPC[g]) * 128
        mat = b2[lo : lo + GPC[g] * 128].reshape(GPC[g], 128).T  # [128, GPC]
        return np.ascontiguousarray(
            np.broadcast_to(mat[:, None, :], (128, 16, GPC[g]))
            .reshape(128, GPC[g] * 16)
            .astype(ndt)
        )

    def wshard(g, c):  # -> [128, GPC[g]*512] in [q, t, l, m] order
        ksg = GPC[g] * 128
        lo = GOFF[g] * 128 + c * ksg
        sh = W2[lo : lo + ksg]
        return (
            sh.reshape(GPC[g], 128, 4, 128)
            .transpose(3, 0, 2, 1)
            .reshape(128, GPC[g] * 512)
        )

    in_maps = []
    for c in range(NCORE):
        wmain = np.ascontiguousarray(
            np.concatenate([wshard(g, c) for g in range(3)], axis=1).astype(ndt)
        )
        xp = np.ascontiguousarray(
            x[c * BC : (c + 1) * BC].reshape(BC * FIN, HWP).astype(ndt)
        )
        in_maps.append(
            {
                "wmain": wmain,
                "latt": latt,
                "b2dup1": b2pre(0, c),
                "b2dup2": b2pre(1, c),
                "b2dup3": b2pre(2, c),
                "xs": xp,
            }
        )
    return in_maps


def _post(raw_all, lat, W, b):
    """raw device outs (list of [BC*FOUT, HWP]) -> full f32 output."""
    out = np.concatenate([np.asarray(r, np.float32) for r in raw_all], axis=0)
    out = out.reshape(B, FOUT, HWP)
    out += _host_bos(lat, W, b)[:, :, None]
    return np.ascontiguousarray(out.reshape(B, FOUT, 128, 128))


def emulate(x, lat, W, b):
    """Numpy emulation of the device dataflow (for layout validation)."""
    xf = np.asarray(x, np.float32).reshape(B, FIN, HWP)
    W2, b2 = _build_w2b2(np.asarray(W, np.float32), np.asarray(b, np.float32))
    ksT = W2 @ np.asarray(lat, np.float32).T + b2[:, None]
    v1 = ksT[: GT[0] * 128].reshape(GT[0], 128, B)
    v2 = ksT[GT[0] * 128 : GOFF[2] * 128].reshape(GT[1], 128, B)
    v3 = ksT[GOFF[2] * 128 :].reshape(GT[2], 128, B)
    raws = []
    for c in range(NCORE):
        raw = np.zeros((BC * FOUT, HWP), np.float32)
        for s in range(BC):
            bi = c * BC + s
            xb = xf[bi]
            h1 = np.maximum(v1[0:128, 0:64, bi] @ xb + v1[G1_BIN, :, bi][:, None], 0.0)
            h2 = np.maximum(v2[0:128, :, bi] @ h1 + v2[G2_BMIDA, :, bi][:, None], 0.0)
            h3 = np.maximum(
                v3[0:128, :, bi] @ h2 + v3[G3_BMIDB, :, bi][:, None], 0.0
            )
            a_out = v3[G3_KOUT : G3_KOUT + 64, :, bi]
            a_sh = v3[G3_KSH : G3_KSH + 64, 0:64, bi]
            raw[s * 64 : (s + 1) * 64] = a_out @ h3 + a_sh @ xb
        raws.append(raw)
    return _post(raws, lat, W, b)


# ---------------------------- bass program ----------------------------------

def _build_nc():
    import concourse.tile as tile
    from concourse import bacc, mybir

    F32 = mybir.dt.float32
    DT = mybir.dt.bfloat16
    AF = mybir.ActivationFunctionType
    ALU = mybir.AluOpType

    nc = bacc.Bacc("TRN2", target_bir_lowering=False, debug=False, num_devices=NCORE)

    wtot = sum(GPC) * 512
    wmain = nc.dram_tensor("wmain", [128, wtot], DT, kind="ExternalInput")
    latt = nc.dram_tensor("latt", [128, 64], DT, kind="ExternalInput")
    b2d = [
        nc.dram_tensor(f"b2dup{g + 1}", [128, GPC[g] * 16], DT, kind="ExternalInput")
        for g in range(3)
    ]
    xs = nc.dram_tensor("xs", [BC * FIN, HWP], DT, kind="ExternalInput")
    outd = nc.dram_tensor("out", [BC * FOUT, HWP], DT, kind="ExternalOutput")

    with tile.TileContext(nc) as tc:
        with (
            tc.tile_pool(name="persist", bufs=1) as pp,
            tc.tile_pool(name="wpool", bufs=3) as wp,
            tc.tile_pool(name="conv", bufs=2) as cp,
            tc.tile_pool(name="ps", bufs=4, space="PSUM") as psp,
            tc.tile_pool(name="dram", bufs=1, space="DRAM") as dp,
        ):
            # ---- phase A: input DMAs (scalar queue: small stuff; gpsimd: x)
            latt_sb = pp.tile([128, 64], DT)
            nc.scalar.dma_start(latt_sb[:], latt[:])
            b2sb = []
            for g in range(3):
                t = pp.tile([128, GPC[g] * 16], DT, name=f"b2sb{g}")
                nc.scalar.dma_start(t[:], b2d[g][:])
                b2sb.append(t)
            xpk = pp.tile([128, HWP], DT, name="xpk")
            for i in range(4):
                (nc.gpsimd if i % 2 == 0 else nc.scalar).dma_start(
                    xpk[:, i * 4096 : (i + 1) * 4096],
                    xs[:, i * 4096 : (i + 1) * 4096],
                )

            wm_off = [0]
            for g in range(3):
                wm_off.append(wm_off[-1] + GPC[g] * 512)

            # ---- phase B: hypernet per group -> sample-major staging
            cc_in, cc_out = [], []
            for g in range(3):
                cc_in.append(
                    dp.tile([NCORE, 128, GPC[g] * BC], DT, name=f"cc_in{g}")
                )
                cc_out.append(
                    dp.tile([NCORE, 128, GPC[g] * BC], DT, name=f"cc_out{g}")
                )

            def hyper_group(g):
                # whole-group staging tile, sample-major [p, (d, s, t)]
                ks_stg = pp.tile([128, GPC[g] * 16], DT, name=f"ks_stg{g}")
                stg_v = ks_stg.rearrange(
                    "p (d s t) -> p d s t", d=NCORE, s=BC, t=GPC[g]
                )
                toff = 0
                for ci, csz in enumerate(GCHUNKS[g]):
                    wt = wp.tile([128, 9 * 512], DT, tag="wt", name=f"wt{g}{ci}")
                    lo = wm_off[g] + toff * 512
                    nc.sync.dma_start(
                        wt[:, 0 : csz * 512], wmain[:, lo : lo + csz * 512]
                    )
                    wtv = wt.rearrange("p (t l m) -> p t l m", t=9, l=4)
                    pk = psp.tile([128, csz * 16], F32, tag="ps", name=f"pk{g}{ci}")
                    for tl in range(csz):
                        for l in range(4):
                            nc.tensor.matmul(
                                pk[:, tl * 16 : (tl + 1) * 16],
                                wtv[:, tl, l, :],
                                latt_sb[:, l * 16 : (l + 1) * 16],
                                start=(l == 0),
                                stop=(l == 3),
                            )
                    b2v = b2sb[g].rearrange(
                        "p (d s t) -> p d s t", d=NCORE, s=BC, t=GPC[g]
                    )
                    nc.vector.tensor_tensor(
                        stg_v[:, :, :, toff : toff + csz],
                        pk.rearrange("p (t d s) -> p d s t", t=csz, d=NCORE, s=BC),
                        b2v[:, :, :, toff : toff + csz],
                        op=ALU.add,
                    )
                    toff += csz
                nc.sync.dma_start(
                    cc_in[g][:, :, :].rearrange("d p r -> p d r"),
                    stg_v.rearrange("p d s t -> p d (s t)"),
                )

            for g in range(3):
                hyper_group(g)

            # ---- phase C: three pipelined AllToAlls (gpsimd triggers)
            for g in range(3):
                nc.gpsimd.collective_compute(
                    "AllToAll",
                    ALU.bypass,
                    replica_groups=[list(range(NCORE))],
                    ins=[cc_in[g].opt()],
                    outs=[cc_out[g].opt()],
                )

            # Post-exchange work is wrapped in tile_wait_until so the Tile
            # scheduler (whose cost model does not know the ~50us CC-init
            # barrier) cannot hoist it ahead of the staging copies/DMAs that
            # feed later AllToAll triggers — that ordering froze the CC
            # pipeline (trigger blocked -> 135us barrier).
            ksr = []
            for g in range(3):
                ksr.append(pp.tile([128, GT[g] * BC], DT, name=f"ksraw{g}"))
            pksh = pp.tile([128, 128], DT, name="pksh")
            nc.vector.memset(pksh[:], 0.0)
            with tc.tile_wait_until(0.060):
                nc.sync.dma_start(
                    ksr[0].rearrange("p (s c t) -> p s c t", s=BC, c=NCORE),
                    cc_out[0][:, :, :].rearrange("c p (s t) -> p s c t", s=BC),
                )
                ks1 = ksr[0]
                bin2 = pp.tile([128, 2], F32, name="bin2")
                nc.vector.tensor_copy(bin2[:, 0:1], ks1[:, G1_BIN : G1_BIN + 1])
                nc.vector.tensor_copy(
                    bin2[:, 1:2], ks1[:, GT[0] + G1_BIN : GT[0] + G1_BIN + 1]
                )
                kin1 = pp.tile([128, 128], DT, name="kin1")
                nc.gpsimd.dma_start(
                    kin1[64:128, 0:128], ks1[0:64, GT[0] : GT[0] + 128]
                )
            with tc.tile_wait_until(0.068):
                nc.gpsimd.dma_start(
                    ksr[1].rearrange("p (s c t) -> p s c t", s=BC, c=NCORE),
                    cc_out[1][:, :, :].rearrange("c p (s t) -> p s c t", s=BC),
                )
                ks2 = ksr[1]
                bma2 = pp.tile([128, 2], F32, name="bma2")
                nc.gpsimd.tensor_copy(bma2[:, 0:1], ks2[:, G2_BMIDA : G2_BMIDA + 1])
                nc.gpsimd.tensor_copy(
                    bma2[:, 1:2], ks2[:, GT[1] + G2_BMIDA : GT[1] + G2_BMIDA + 1]
                )
            with tc.tile_wait_until(0.076):
                nc.gpsimd.dma_start(
                    ksr[2].rearrange("p (s c t) -> p s c t", s=BC, c=NCORE),
                    cc_out[2][:, :, :].rearrange("c p (s t) -> p s c t", s=BC),
                )
                ks3 = ksr[2]
                bmb2 = pp.tile([128, 2], F32, name="bmb2")
                nc.gpsimd.tensor_copy(bmb2[:, 0:1], ks3[:, G3_BMIDB : G3_BMIDB + 1])
                nc.gpsimd.tensor_copy(
                    bmb2[:, 1:2], ks3[:, GT[2] + G3_BMIDB : GT[2] + G3_BMIDB + 1]
                )
                nc.gpsimd.dma_start(
                    pksh[0:64, 0:64], ks3[0:64, G3_KSH : G3_KSH + 64]
                )
                nc.gpsimd.dma_start(
                    pksh[64:128, 64:128],
                    ks3[0:64, GT[2] + G3_KSH : GT[2] + G3_KSH + 64],
                )

            # ---- phase D: conv, chunk-granular software pipeline
            ta, td = [0.0], [0.0]

            def act_op(dst, src, bias, kind, n):
                ca, cd = 0.175 + n / 1.2e3, 0.170 + n / 0.96e3
                if ta[0] + ca <= td[0] + cd:
                    ta[0] += ca
                    if kind == "relu":
                        nc.scalar.activation(dst, src, AF.Relu, bias=bias)
                    else:
                        nc.scalar.activation(dst, src, AF.Copy)
                else:
                    td[0] += cd
                    if kind == "relu":
                        nc.vector.tensor_scalar(
                            dst, src, bias, 0.0, op0=ALU.add, op1=ALU.max
                        )
                    else:
                        nc.vector.tensor_copy(dst, src)

            def layer(dst_h, lhsT, rhs_tile, rows, cols0, bias):
                for half in range(JP // NP):
                    ph = psp.tile([128, NP], F32, tag="ps", name=f"ph{half}")
                    for n in range(NP // MMN):
                        lo = cols0 + half * NP + n * MMN
                        nc.tensor.matmul(
                            ph[:, n * MMN : (n + 1) * MMN],
                            lhsT,
                            rhs_tile[rows, lo : lo + MMN],
                            start=True,
                            stop=True,
                        )
                    hsl = slice(half * NP, (half + 1) * NP)
                    act_op(dst_h[:, hsl], ph[:, 0:NP], bias, "relu", NP)

            HB2 = 10
            h1s, h2s, h3s = {}, {}, {}

            def do_L1(j):
                for s in range(BC):
                    h1 = cp.tile([128, JP], DT, tag="h", bufs=HB2, name=f"h1_{s}")
                    if s == 0:
                        layer(h1, ks1[0:64, 0:128], xpk, slice(0, 64), j * JP,
                              bin2[:, 0:1])
                    else:
                        layer(h1, kin1[64:128, 0:128], xpk, slice(64, 128), j * JP,
                              bin2[:, 1:2])
                    h1s[j, s] = h1

            def do_L2(j):
                for s in range(BC):
                    h2 = cp.tile([128, JP], DT, tag="h", bufs=HB2, name=f"h2_{s}")
                    layer(h2, ks2[:, s * GT[1] : s * GT[1] + 128], h1s[j, s],
                          slice(0, 128), 0, bma2[:, s : s + 1])
                    h2s[j, s] = h2
                del h1s[j, 0], h1s[j, 1]

            def do_L3(j):
                for s in range(BC):
                    h3 = cp.tile([128, JP], DT, tag="h", bufs=HB2, name=f"h3_{s}")
                    layer(h3, ks3[:, s * GT[2] : s * GT[2] + 128], h2s[j, s],
                          slice(0, 128), 0, bmb2[:, s : s + 1])
                    h3s[j, s] = h3
                del h2s[j, 0], h2s[j, 1]

            def do_L4(j):
                oc = cp.tile([128, JP], DT, tag="oc", bufs=3, name="oc")
                for half in range(JP // NP):
                    po = psp.tile([128, NP], F32, tag="ps", name=f"po{half}")
                    for n in range(NP // MMN):
                        nsl = slice(n * MMN, (n + 1) * MMN)
                        hlo = half * NP + n * MMN
                        nc.tensor.matmul(
                            po[:, nsl],
                            pksh[:],
                            xpk[:, j * JP + hlo : j * JP + hlo + MMN],
                            start=True,
                            stop=False,
                        )
                        nc.tensor.matmul(
                            po[0:64, nsl],
                            ks3[:, G3_KOUT : G3_KOUT + 64],
                            h3s[j, 0][:, hlo : hlo + MMN],
                            start=False,
                            stop=False,
                            tile_position=(0, 0),
                        )
                        nc.tensor.matmul(
                            po[64:128, nsl],
                            ks3[:, GT[2] + G3_KOUT : GT[2] + G3_KOUT + 64],
                            h3s[j, 1][:, hlo : hlo + MMN],
                            start=False,
                            stop=True,
                            tile_position=(0, 64),
                        )
                    act_op(oc[:, half * NP : (half + 1) * NP], po[:, 0:NP],
                           None, "copy", NP)
                nc.gpsimd.dma_start(outd[:, j * JP : (j + 1) * JP], oc[:])
                del h3s[j, 0], h3s[j, 1]

            for r in range(NJ + 3):
                with tc.tile_wait_until(0.062 + 0.004 * r):
                    if r < NJ:
                        do_L1(r)
                    if 0 <= r - 1 < NJ:
                        do_L2(r - 1)
                    if 0 <= r - 2 < NJ:
                        do_L3(r - 2)
                    if 0 <= r - 3 < NJ:
                        do_L4(r - 3)

    nc.compile()
    return nc


_NC_CACHE = None


def kernel(x, lat, W, b):
    from concourse.bass_utils import run_bass_kernel_spmd

    global _NC_CACHE
    if _NC_CACHE is None:
        _NC_CACHE = _build_nc()
    nc = _NC_CACHE
    in_maps = _host_inputs(x, lat, W, b)
    res = run_bass_kernel_spmd(nc, in_maps, core_ids=list(range(NCORE)))
    raws = [res.results[c]["out"] for c in range(NCORE)]
    return _post(raws, lat, W, b)
